# revision 1
# baseline (speedup 1.0000x reference)
"""Trainium2 kernel for nn_Eq2Net_7859790151696.

Device (8 NeuronCores, SPMD, t-sharded): the head projections
logits = s_i @ [W_action | W_stop | W_start]  -- all of the input memory
traffic (s_i is 4.2 MB of the 4.85 MB total) and virtually all FLOPs.
Each core computes a 257-row t-shard of the (2049, 336) logits.

Host: the strictly-sequential T=2048, B=16 HMM recurrence, reformulated as a
chunked linear solve (validated to ~5e-7 rel err against the jax reference):
the (T,B) log-buffer collapses to Ut_i = (D_i + a_i s_i^T) Ut_{i-1} in prob
space; the scalar rearrange flux p satisfies p = c + K p with K = tril(alpha
beta^T, -1) rank-16; solved per 128-chunk with a nilpotent doubling inverse
and cross-chunk 16-dim state with rescaling. O(T*B + NC*L^2) host work on
tiny data (the sequential part is irreducible on any backend).
"""
import numpy as np

T, S, B, A = 2048, 512, 16, 18
PEN = 0.5
NCORES = 8
ROWS = 257          # 2049 rows padded to 8*257 = 2056
NPAD = 8 * ROWS
L, NCHUNK = 128, 16

_prog = None


def _build_program():
    import concourse.bass as bass
    import concourse.tile as tile
    from concourse import bacc, mybir

    nc = bacc.Bacc("TRN2", target_bir_lowering=False, debug=False,
                   num_devices=NCORES)
    # bf16 I/O: host<->device transfer over the axon tunnel dominates wall
    # time; PE matmuls bf16 natively with fp32 PSUM accumulation.
    sT = nc.dram_tensor("sT", [S, ROWS], mybir.dt.bfloat16,
                        kind="ExternalInput")
    W = nc.dram_tensor("W", [S, 336], mybir.dt.bfloat16,
                       kind="ExternalInput")
    out = nc.dram_tensor("logits", [ROWS, 336], mybir.dt.bfloat16,
                         kind="ExternalOutput")

    with tile.TileContext(nc) as tc:
        with tc.tile_pool(name="sb", bufs=1) as pool, \
             tc.tile_pool(name="ps", bufs=2, space="PSUM") as pps:
            # plain 2D DMAs, each staged through one compute op so downstream
            # matmuls wait on a single semaphore (walrus caps sync waits per
            # instruction and a wide DMA fans out over many DGE queues)
            sT_sb = pool.tile([128, 4, ROWS], mybir.dt.bfloat16, tag="sT")
            W_sb = pool.tile([128, 4, 336], mybir.dt.bfloat16, tag="W")
            for k in range(4):
                tr = pool.tile([128, ROWS], mybir.dt.bfloat16, tag=f"sTr{k}")
                nc.gpsimd.dma_start(tr[:], sT[k * 128:(k + 1) * 128, :])
                nc.scalar.copy(sT_sb[:, k, :], tr[:])
                wr = pool.tile([128, 336], mybir.dt.bfloat16, tag=f"Wr{k}")
                nc.gpsimd.dma_start(wr[:], W[k * 128:(k + 1) * 128, :])
                nc.scalar.copy(W_sb[:, k, :], wr[:])
            for m, mlen in ((0, 128), (128, 128), (256, 1)):
                ps = pps.tile([mlen, 336], mybir.dt.float32, tag=f"ps{m}")
                for k in range(4):
                    nc.tensor.matmul(ps[:], sT_sb[:, k, m:m + mlen],
                                     W_sb[:, k, :], start=(k == 0),
                                     stop=(k == 3))
                ot = pool.tile([mlen, 336], mybir.dt.bfloat16, tag=f"ot{m}")
                nc.scalar.copy(ot[:], ps[:])
                nc.gpsimd.dma_start(out[m:m + mlen, :], ot[:])
    nc.compile()
    return nc


def _run_device(s_i, Wcat):
    global _prog
    if _prog is None:
        _prog = _build_program()
    import ml_dtypes
    from concourse.bass_utils import run_bass_kernel_spmd
    bf16 = ml_dtypes.bfloat16
    Wb = np.ascontiguousarray(Wcat.astype(bf16))
    in_maps = []
    for c in range(NCORES):
        r0 = c * ROWS
        nrows = min(ROWS, T + 1 - r0)             # last shard is 250 rows
        shard = np.zeros((S, ROWS), bf16)
        shard[:, :nrows] = s_i[r0:r0 + nrows].astype(bf16).T
        in_maps.append({"sT": shard, "W": Wb})
    res = run_bass_kernel_spmd(_prog, in_maps, core_ids=list(range(NCORES)))
    logits = np.concatenate([res.results[c]["logits"] for c in range(NCORES)],
                            axis=0)[:T + 1]
    return logits


def _host_scan(logits, actions):
    f32 = np.float32
    la = logits[:, :288].reshape(T + 1, B, A)
    lst = logits[:, 288:320].reshape(T + 1, B, 2)
    lsr = logits[:, 320:336]
    act = np.asarray(actions).astype(np.int64)
    # heads (bounded logits: no max-shift needed)
    ea = np.exp(la)
    e = (ea[np.arange(T)[:, None], np.arange(B)[None, :], act[:, None]]
         / ea[:T].sum(-1)).astype(f32)
    delta = (lst[:, :, 0] - lst[:, :, 1]).astype(f32)
    expm = np.exp(-delta)
    ds = (1.0 / (1.0 + expm)).astype(f32)
    ss = (expm * ds).astype(f32)
    ld = (-np.log1p(expm)).astype(f32)
    er = np.exp(lsr)
    at = (np.exp(f32(-PEN)) * er / er.sum(-1, keepdims=True)).astype(f32)

    ld = ld.copy()
    ld[0] = 0.0
    C = np.cumsum(ld[:T], 0, dtype=f32)          # C_i global, i=0..T-1
    tril = np.tril(np.ones((L, L), f32), -1)
    tot = 0.0
    logscale = 0.0
    lam_sum = 0.0
    zrow = None
    aux = []
    for c in range(NCHUNK):
        i0 = c * L
        Cl = C[i0:i0 + L]
        Cstart = C[i0 - 1] if c > 0 else np.zeros(B, f32)
        Cm = (0.5 * (Cstart + Cl[-1])).astype(f32)
        Clprev = np.vstack([Cstart, Cl[:-1]])
        alpha = ss[i0:i0 + L] * np.exp(Clprev - Cm)
        beta = at[i0:i0 + L] * np.exp(Cm - Cl)
        if c == 0:
            alpha[0] = 0.0
            beta[0] = 0.0
        K = np.where(tril > 0, alpha @ beta.T, f32(0))
        SA = alpha.copy()
        Ks = K
        for s in range(7):                        # exact: K^0..K^127
            SA = SA + Ks @ SA
            if s < 6:
                Ks = Ks @ Ks
        aux.append((Cl, Cm, beta, SA))
    for c in range(NCHUNK):
        i0 = c * L
        Cl, Cm, beta, SA = aux[c]
        if c == 0:
            zhat = (np.exp(lsr[0]) / np.exp(lsr[0]).sum()
                    * np.exp(Cm)).astype(f32)
        p = SA @ zhat
        Y = zhat[None, :] + np.cumsum(beta * p[:, None], 0, dtype=f32)
        w = ((e[i0:i0 + L] * np.exp(Cl - Cm)) * Y).sum(1)
        tot += np.log(w).sum() + L * logscale
        zend = np.exp(Cl[-1] - Cm) * Y[-1]
        if c < NCHUNK - 1:
            mu = zend.sum()
            zhat = ((zend / mu) * np.exp(aux[c + 1][1] - Cl[-1])).astype(f32)
            logscale += np.log(mu)
    tot += np.log((ds[T] * zend).sum()) + logscale
    return np.float32(tot)


def kernel(s_i, W_action, W_stop, W_start, actions):
    s_i = np.asarray(s_i, np.float32)
    Wcat = np.ascontiguousarray(
        np.concatenate([np.asarray(W_action, np.float32),
                        np.asarray(W_stop, np.float32),
                        np.asarray(W_start, np.float32)], axis=1))
    logits = _run_device(s_i, Wcat)
    return _host_scan(logits.astype(np.float32), actions)



# revision 2
# speedup vs baseline: 4.6200x; 4.6200x over previous
"""Trainium2 kernel for nn_Eq2Net_7859790151696.

The reference's O(T^2 * B) log-space buffer recurrence collapses exactly to a
B=16 linear recurrence in probability space:

    p_i = c_i * p_{i-1} + kappa * s'_i * (z_i . p_{i-1})        (rank-1 update)
    d_i = a_i . p_i ;  p_i /= d_i                               (per-step norm)
    total = sum_j (T+1-j) * ln d_j + ln(c_T . p_final)

where c/z are the stop-head sigmoids, s' the start-head softmax, a the action
prob of the taken action. Everything (fp8 matmul of the heads, softmaxes, the
T=2048-step sequential scan at 5 DVE instructions/step, and the final weighted
log-sum) runs in ONE single-core device launch that returns one f32 scalar, so
per-call wall time is dominated by the fixed axon round trip. Inputs ship as
fp8 (s_i, 64*W) + bf16 one-hot actions (~1.3 MB); validated rel err ~1e-4.
"""
import numpy as np
import ml_dtypes

T, S, B, A = 2048, 512, 16, 18
PEN = 0.5
KAPPA = float(np.exp(np.float32(-PEN)))
NROW = T + 1            # 2049
NT = 17                 # 16 tiles of 128 rows + 1 tile of 1 row (row 2048)
CHUNK = 256
NCHUNK = T // CHUNK     # 8
FP8 = ml_dtypes.float8_e4m3
BF16 = ml_dtypes.bfloat16

_runner = None


def _build_program():
    import concourse.bass as bass  # noqa
    import concourse.tile as tile
    from concourse import bacc, mybir

    nc = bacc.Bacc("TRN2", target_bir_lowering=False, debug=False,
                   num_devices=1)
    f32 = mybir.dt.float32
    fp8 = mybir.dt.float8e4
    bf16 = mybir.dt.bfloat16
    AF = mybir.ActivationFunctionType
    OP = mybir.AluOpType
    AX = mybir.AxisListType

    sT8 = nc.dram_tensor("sT8", [S, NROW], fp8, kind="ExternalInput")
    W8 = nc.dram_tensor("W8", [S, 336], fp8, kind="ExternalInput")
    OH = nc.dram_tensor("OH", [T, A], bf16, kind="ExternalInput")
    out = nc.dram_tensor("out", [1, 1], f32, kind="ExternalOutput")

    with tile.TileContext(nc) as tc:
        with tc.tile_pool(name="dram", bufs=1, space="DRAM") as dpool, \
             tc.tile_pool(name="cst", bufs=1) as cpool, \
             tc.tile_pool(name="sb", bufs=2) as pool, \
             tc.tile_pool(name="ps", bufs=2, space="PSUM") as pps:
            # DRAM scratch for per-step head probabilities (row-major (t, b))
            Cd = dpool.tile([NROW, B], f32, tag="Cd")    # sigmoid(delta)
            Zd = dpool.tile([T, B], f32, tag="Zd")       # sigmoid(-delta)
            Sd = dpool.tile([T, B], f32, tag="Sd")       # kappa*softmax(start)
            Ad = dpool.tile([T, B], f32, tag="Ad")       # taken-action prob

            # ---- load inputs (staged through one copy per chunk) ----
            sT_sb = cpool.tile([128, 4, NROW], fp8, tag="sT")
            W_sb = cpool.tile([128, 4, 336], fp8, tag="W")
            for k in range(4):
                tr = pool.tile([128, NROW], fp8, tag="sTr")
                nc.sync.dma_start(tr[:], sT8[k * 128:(k + 1) * 128, :])
                nc.scalar.copy(sT_sb[:, k, :], tr[:])
                wr = pool.tile([128, 336], fp8, tag="Wr")
                nc.sync.dma_start(wr[:], W8[k * 128:(k + 1) * 128, :])
                nc.scalar.copy(W_sb[:, k, :], wr[:])

            dsub_sb = cpool.tile([128, NT, B], f32, tag="dsub")

            # ---- per-row-tile: matmul + exp-based heads (Sigmoid deferred so
            # the ACT table set never thrashes; Copy is in every set) ----
            for t in range(NT):
                m0 = t * 128
                mlen = min(128, NROW - m0)
                ps = pps.tile([mlen, 336], f32, tag="ps")
                for k in range(4):
                    nc.tensor.matmul(ps[:], sT_sb[:, k, m0:m0 + mlen],
                                     W_sb[:, k, :], start=(k == 0),
                                     stop=(k == 3))
                lg = pool.tile([mlen, 336], f32, tag="lg")
                nc.scalar.mul(lg[:], ps[:], 1.0 / 64.0)
                stopv = lg[:, 288:320].rearrange("p (b two) -> p b two", two=2)
                nc.vector.tensor_tensor(dsub_sb[:mlen, t, :], stopv[:, :, 0],
                                        stopv[:, :, 1], op=OP.subtract)
                if t == NT - 1:
                    continue  # row 2048: only the final stop prob is needed
                # action head
                ea = pool.tile([mlen, 288], f32, tag="ea")
                nc.scalar.activation(ea[:], lg[:, 0:288], AF.Exp)
                eav = ea[:].rearrange("p (b a) -> p b a", a=A)
                den = pool.tile([mlen, B], f32, tag="den")
                nc.vector.tensor_reduce(den[:], eav, axis=AX.X, op=OP.add)
                oh_t = pool.tile([mlen, A], bf16, tag="oh")
                nc.sync.dma_start(oh_t[:], OH[m0:m0 + mlen, :])
                tmp = pool.tile([mlen, B, A], f32, tag="tmp")
                num = pool.tile([mlen, B], f32, tag="num")
                nc.vector.tensor_tensor(
                    tmp[:], eav, oh_t[:].unsqueeze(1).broadcast_to([mlen, B, A]),
                    op=OP.mult)
                nc.vector.tensor_reduce(num[:], tmp[:], axis=AX.X, op=OP.add)
                rden = pool.tile([mlen, B], f32, tag="rden")
                nc.vector.reciprocal(rden[:], den[:])
                a_t = pool.tile([mlen, B], f32, tag="a_t")
                nc.vector.tensor_tensor(a_t[:], num[:], rden[:], op=OP.mult)
                nc.sync.dma_start(Ad[m0:m0 + mlen, :], a_t[:])
                # start head
                es = pool.tile([mlen, B], f32, tag="es")
                esum = pool.tile([mlen, 1], f32, tag="esum")
                nc.scalar.activation(es[:], lg[:, 320:336], AF.Exp,
                                     accum_out=esum[:])
                resum = pool.tile([mlen, 1], f32, tag="resum")
                nc.vector.reciprocal(resum[:], esum[:])
                spp_t = pool.tile([mlen, B], f32, tag="spp")
                nc.vector.tensor_scalar(spp_t[:], es[:], resum[:], KAPPA,
                                        op0=OP.mult, op1=OP.mult)
                nc.sync.dma_start(Sd[m0:m0 + mlen, :], spp_t[:])

            # ---- sigmoid pass (single ACT table switch) ----
            for t in range(NT):
                m0 = t * 128
                mlen = min(128, NROW - m0)
                c_t = pool.tile([mlen, B], f32, tag="c_t")
                nc.scalar.activation(c_t[:], dsub_sb[:mlen, t, :], AF.Sigmoid)
                nc.sync.dma_start(Cd[m0:m0 + mlen, :], c_t[:])
                if t == NT - 1:
                    continue
                z_t = pool.tile([mlen, B], f32, tag="z_t")
                nc.scalar.activation(z_t[:], dsub_sb[:mlen, t, :], AF.Sigmoid,
                                     scale=-1.0)
                nc.sync.dma_start(Zd[m0:m0 + mlen, :], z_t[:])

            # ---- sequential scan on partition 0: 5 DVE instrs/step ----
            ph = cpool.tile([1, B], f32, tag="ph")      # unnormalized p-hat
            cq = cpool.tile([1, B], f32, tag="cq")
            jk = cpool.tile([1, B], f32, tag="jk")      # junk elementwise out
            mm = cpool.tile([1, 1], f32, tag="mm")
            rr = cpool.tile([1, 1], f32, tag="rr")
            dv = cpool.tile([1, T], f32, tag="dv")      # per-step d values

            for ch in range(NCHUNK):
                r0 = ch * CHUNK
                Cb = pool.tile([1, CHUNK * B], f32, tag="Cb")
                Zb = pool.tile([1, CHUNK * B], f32, tag="Zb")
                Sb = pool.tile([1, CHUNK * B], f32, tag="Sb")
                Ab = pool.tile([1, CHUNK * B], f32, tag="Ab")
                nc.sync.dma_start(Cb[:], Cd[r0:r0 + CHUNK, :])
                nc.sync.dma_start(Zb[:], Zd[r0:r0 + CHUNK, :])
                nc.sync.dma_start(Sb[:], Sd[r0:r0 + CHUNK, :])
                nc.sync.dma_start(Ab[:], Ad[r0:r0 + CHUNK, :])
                lstart = 0
                if ch == 0:
                    # step 0: p = softmax(start row 0) = spp row0 / kappa
                    nc.vector.tensor_scalar_mul(ph[:], Sb[0:1, 0:B],
                                                1.0 / KAPPA)
                    nc.vector.scalar_tensor_tensor(
                        jk[:], Ab[0:1, 0:B], 1.0, ph[:],
                        op0=OP.mult, op1=OP.mult, accum_out=dv[0:1, 0:1])
                    nc.vector.reciprocal(rr[:], dv[0:1, 0:1])
                    lstart = 1
                for l in range(lstart, CHUNK):
                    i = r0 + l
                    o = l * B
                    nc.vector.scalar_tensor_tensor(
                        jk[:], Zb[0:1, o:o + B], rr[0:1, 0:1], ph[:],
                        op0=OP.mult, op1=OP.mult, accum_out=mm[:])
                    nc.vector.scalar_tensor_tensor(
                        cq[:], Cb[0:1, o:o + B], rr[0:1, 0:1], ph[:],
                        op0=OP.mult, op1=OP.mult)
                    nc.vector.scalar_tensor_tensor(
                        ph[:], Sb[0:1, o:o + B], mm[0:1, 0:1], cq[:],
                        op0=OP.mult, op1=OP.add)
                    nc.vector.scalar_tensor_tensor(
                        jk[:], Ab[0:1, o:o + B], 1.0, ph[:],
                        op0=OP.mult, op1=OP.mult, accum_out=dv[0:1, i:i + 1])
                    nc.vector.reciprocal(rr[:], dv[0:1, i:i + 1])

            # ---- final: total = sum_j (T+1-j) ln d_j + ln(c_T . p / d_last)
            cT = cpool.tile([1, B], f32, tag="cT")
            nc.sync.dma_start(cT[:], Cd[T:T + 1, :])
            Fv = cpool.tile([1, 1], f32, tag="Fv")
            nc.vector.scalar_tensor_tensor(
                jk[:], cT[:], rr[0:1, 0:1], ph[:],
                op0=OP.mult, op1=OP.mult, accum_out=Fv[:])
            ld = cpool.tile([1, T], f32, tag="ld")
            nc.scalar.activation(ld[:], dv[:], AF.Ln)
            lF = cpool.tile([1, 1], f32, tag="lF")
            nc.scalar.activation(lF[:], Fv[:], AF.Ln)
            wi = cpool.tile([1, T], mybir.dt.int32, tag="wi")
            nc.gpsimd.iota(wi[:], pattern=[[-1, T]], base=T + 1,
                           channel_multiplier=0)
            wf = cpool.tile([1, T], f32, tag="wf")
            nc.vector.tensor_copy(wf[:], wi[:])
            wd = cpool.tile([1, T], f32, tag="wd")
            nc.vector.tensor_tensor(wd[:], ld[:], wf[:], op=OP.mult)
            S1 = cpool.tile([1, 1], f32, tag="S1")
            nc.vector.tensor_reduce(S1[:], wd[:], axis=AX.X, op=OP.add)
            tot = cpool.tile([1, 1], f32, tag="tot")
            nc.vector.tensor_tensor(tot[:], S1[:], lF[:], op=OP.add)
            nc.sync.dma_start(out[:], tot[:])
    nc.compile()
    return nc


def _make_runner():
    """Build the program once and wrap it in a persistent jitted callable so
    warm calls skip XLA re-trace/re-lowering (run_bass_kernel_spmd rebuilds
    its jit on every call, which costs >100 ms under axon)."""
    import jax
    from concourse import bass2jax as b2j
    from concourse import mybir

    nc = _build_program()
    b2j.install_neuronx_cc_hook()
    partition_name = (nc.partition_id_tensor.name
                      if nc.partition_id_tensor else None)
    in_names, out_names, out_avals, zero_outs = [], [], [], []
    for alloc in nc.m.functions[0].allocations:
        if not isinstance(alloc, mybir.MemoryLocationSet):
            continue
        name = alloc.memorylocations[0].name
        if alloc.kind == "ExternalInput":
            if name != partition_name:
                in_names.append(name)
        elif alloc.kind == "ExternalOutput":
            out_names.append(name)
            shape = tuple(alloc.tensor_shape)
            dtype = mybir.dt.np(alloc.dtype)
            out_avals.append(jax.core.ShapedArray(shape, dtype))
            zero_outs.append(np.zeros(shape, dtype))
    n_params = len(in_names)
    in_names_all = list(in_names) + out_names + (
        [partition_name] if partition_name else [])
    donate = tuple(range(n_params, n_params + len(out_avals)))

    def _body(*args):
        operands = list(args)
        if partition_name is not None:
            operands.append(b2j.partition_id_tensor())
        return tuple(b2j._bass_exec_p.bind(
            *operands, out_avals=tuple(out_avals),
            in_names=tuple(in_names_all), out_names=tuple(out_names),
            lowering_input_output_aliases=(), sim_require_finite=True,
            sim_require_nnan=True, nc=nc))

    jitted = jax.jit(_body, donate_argnums=donate, keep_unused=True)

    def run(in_map):
        args = [np.asarray(in_map[n]) for n in in_names]
        zeros = [np.zeros(z.shape, z.dtype) for z in zero_outs]
        outs = jitted(*args, *zeros)
        return {name: np.asarray(outs[i]) for i, name in enumerate(out_names)}

    return run


def kernel(s_i, W_action, W_stop, W_start, actions):
    global _runner
    if _runner is None:
        _runner = _make_runner()
    s = np.asarray(s_i, np.float32)
    Wcat = np.concatenate([np.asarray(W_action, np.float32),
                           np.asarray(W_stop, np.float32),
                           np.asarray(W_start, np.float32)], axis=1)
    sT8 = np.ascontiguousarray(s.T).astype(FP8)
    W8 = np.ascontiguousarray(Wcat * 64.0).astype(FP8)
    acts = np.asarray(actions).astype(np.int64).reshape(-1)
    OHm = np.zeros((T, A), BF16)
    OHm[np.arange(T), acts] = 1
    res = _runner({"sT8": sT8, "W8": W8, "OH": OHm})
    return np.float32(res["out"].reshape(-1)[0])


# revision 6
# speedup vs baseline: 6.0232x; 1.3037x over previous
"""Trainium2 kernel for nn_Eq2Net_7859790151696.

The reference's O(T^2 * B) log-space buffer recurrence collapses exactly to a
B=16 linear recurrence in probability space:

    p_i = c_i * p_{i-1} + kappa * s'_i * (z_i . p_{i-1})        (rank-1 update)
    d_i = a_i . p_i ;  p_i /= d_i                               (per-step norm)
    total = sum_j (T+1-j) * ln d_j + ln(c_T . p_final)

where c/z are the stop-head sigmoids, s' the start-head softmax, a the action
prob of the taken action. Everything (fp8 matmul of the heads, softmaxes, the
T=2048-step sequential scan at 5 DVE instructions/step, and the final weighted
log-sum) runs in ONE single-core device launch that returns one f32 scalar, so
per-call wall time is dominated by the fixed axon round trip. Inputs ship as
fp8 (s_i, 64*W) + bf16 one-hot actions (~1.3 MB); validated rel err ~1e-4.
"""
import numpy as np
import ml_dtypes

T, S, B, A = 2048, 512, 16, 18
PEN = 0.5
KAPPA = float(np.exp(np.float32(-PEN)))
NROW = T + 1            # 2049
NT = 17                 # 16 tiles of 128 rows + 1 tile of 1 row (row 2048)
CHUNK = 256
NCHUNK = T // CHUNK     # 8
FP8 = ml_dtypes.float8_e4m3
BF16 = ml_dtypes.bfloat16

_runner = None


def _build_program():
    import concourse.bass as bass  # noqa
    import concourse.tile as tile
    from concourse import bacc, mybir

    nc = bacc.Bacc("TRN2", target_bir_lowering=False, debug=False,
                   num_devices=1)
    f32 = mybir.dt.float32
    fp8 = mybir.dt.float8e4
    bf16 = mybir.dt.bfloat16
    AF = mybir.ActivationFunctionType
    OP = mybir.AluOpType
    AX = mybir.AxisListType

    s8 = nc.dram_tensor("s8", [NROW, S], fp8, kind="ExternalInput")
    W8 = nc.dram_tensor("W8", [S, 336], fp8, kind="ExternalInput")
    OH = nc.dram_tensor("OH", [T, A], bf16, kind="ExternalInput")
    out = nc.dram_tensor("out", [1, 1], f32, kind="ExternalOutput")
    from concourse.masks import make_identity

    with tile.TileContext(nc) as tc:
        with tc.tile_pool(name="dram", bufs=1, space="DRAM") as dpool, \
             tc.tile_pool(name="cst", bufs=1) as cpool, \
             tc.tile_pool(name="sb", bufs=2) as pool, \
             tc.tile_pool(name="ps", bufs=2, space="PSUM") as pps:
            # DRAM scratch for per-step head probabilities (row-major (t, b))
            Cd = dpool.tile([NROW, B], f32, tag="Cd")    # sigmoid(delta)
            Zd = dpool.tile([T, B], f32, tag="Zd")       # sigmoid(-delta)
            Sd = dpool.tile([T, B], f32, tag="Sd")       # kappa*softmax(start)
            Ad = dpool.tile([T, B], f32, tag="Ad")       # taken-action prob

            # ---- load W (staged through one copy per chunk) ----
            sT_sb = cpool.tile([128, 4, NROW], bf16, tag="sT")
            W_sb = cpool.tile([128, 4, 336], bf16, tag="W")
            ident = cpool.tile([128, 128], bf16, tag="ident")
            make_identity(nc, ident[:])
            for k in range(4):
                wr = pool.tile([128, 336], fp8, tag="Wr")
                nc.sync.dma_start(wr[:], W8[k * 128:(k + 1) * 128, :])
                nc.scalar.copy(W_sb[:, k, :], wr[:])

            dsub_sb = cpool.tile([128, NT, B], f32, tag="dsub")

            # ---- per-row-tile: on-device transpose of s (PE via identity),
            # matmul + exp-based heads (Sigmoid deferred so the ACT table set
            # never thrashes; Copy is in every set) ----
            for t in range(NT):
                m0 = t * 128
                mlen = min(128, NROW - m0)
                srow = pool.tile([mlen, S], fp8, tag="srow")
                nc.sync.dma_start(srow[:], s8[m0:m0 + mlen, :])
                srow16 = pool.tile([mlen, S], bf16, tag="srow16")
                nc.scalar.copy(srow16[:], srow[:])
                for k in range(4):
                    tp = pps.tile([128, mlen], bf16, tag="tp")
                    nc.tensor.transpose(tp[:], srow16[:, k * 128:(k + 1) * 128],
                                        ident[:mlen, :mlen])
                    nc.scalar.copy(sT_sb[:, k, m0:m0 + mlen], tp[:])
                ps = pps.tile([mlen, 336], f32, tag="ps")
                for k in range(4):
                    nc.tensor.matmul(ps[:], sT_sb[:, k, m0:m0 + mlen],
                                     W_sb[:, k, :], start=(k == 0),
                                     stop=(k == 3))
                lg = pool.tile([mlen, 336], f32, tag="lg")
                nc.scalar.mul(lg[:], ps[:], 1.0 / 64.0)
                stopv = lg[:, 288:320].rearrange("p (b two) -> p b two", two=2)
                nc.vector.tensor_tensor(dsub_sb[:mlen, t, :], stopv[:, :, 0],
                                        stopv[:, :, 1], op=OP.subtract)
                if t == NT - 1:
                    continue  # row 2048: only the final stop prob is needed
                # action head
                ea = pool.tile([mlen, 288], f32, tag="ea")
                nc.scalar.activation(ea[:], lg[:, 0:288], AF.Exp)
                eav = ea[:].rearrange("p (b a) -> p b a", a=A)
                den = pool.tile([mlen, B], f32, tag="den")
                nc.vector.tensor_reduce(den[:], eav, axis=AX.X, op=OP.add)
                oh_t = pool.tile([mlen, A], bf16, tag="oh")
                nc.sync.dma_start(oh_t[:], OH[m0:m0 + mlen, :])
                tmp = pool.tile([mlen, B, A], f32, tag="tmp")
                num = pool.tile([mlen, B], f32, tag="num")
                nc.vector.tensor_tensor(
                    tmp[:], eav, oh_t[:].unsqueeze(1).broadcast_to([mlen, B, A]),
                    op=OP.mult)
                nc.vector.tensor_reduce(num[:], tmp[:], axis=AX.X, op=OP.add)
                rden = pool.tile([mlen, B], f32, tag="rden")
                nc.vector.reciprocal(rden[:], den[:])
                a_t = pool.tile([mlen, B], f32, tag="a_t")
                nc.vector.tensor_tensor(a_t[:], num[:], rden[:], op=OP.mult)
                nc.sync.dma_start(Ad[m0:m0 + mlen, :], a_t[:])
                # start head
                es = pool.tile([mlen, B], f32, tag="es")
                esum = pool.tile([mlen, 1], f32, tag="esum")
                nc.scalar.activation(es[:], lg[:, 320:336], AF.Exp,
                                     accum_out=esum[:])
                resum = pool.tile([mlen, 1], f32, tag="resum")
                nc.vector.reciprocal(resum[:], esum[:])
                spp_t = pool.tile([mlen, B], f32, tag="spp")
                nc.vector.tensor_scalar(spp_t[:], es[:], resum[:], KAPPA,
                                        op0=OP.mult, op1=OP.mult)
                nc.sync.dma_start(Sd[m0:m0 + mlen, :], spp_t[:])

            # ---- sigmoid pass (single ACT table switch) ----
            for t in range(NT):
                m0 = t * 128
                mlen = min(128, NROW - m0)
                c_t = pool.tile([mlen, B], f32, tag="c_t")
                nc.scalar.activation(c_t[:], dsub_sb[:mlen, t, :], AF.Sigmoid)
                nc.sync.dma_start(Cd[m0:m0 + mlen, :], c_t[:])
                if t == NT - 1:
                    continue
                z_t = pool.tile([mlen, B], f32, tag="z_t")
                nc.scalar.activation(z_t[:], dsub_sb[:mlen, t, :], AF.Sigmoid,
                                     scale=-1.0)
                nc.sync.dma_start(Zd[m0:m0 + mlen, :], z_t[:])

            # ---- sequential scan on partition 0: 5 DVE instrs/step ----
            ph = cpool.tile([1, B], f32, tag="ph")      # unnormalized p-hat
            cq = cpool.tile([1, B], f32, tag="cq")
            jk = cpool.tile([1, B], f32, tag="jk")      # junk elementwise out
            mm = cpool.tile([1, 1], f32, tag="mm")
            rr = cpool.tile([1, 1], f32, tag="rr")
            dv = cpool.tile([1, T], f32, tag="dv")      # per-step d values

            for ch in range(NCHUNK):
                r0 = ch * CHUNK
                Cb = pool.tile([1, CHUNK * B], f32, tag="Cb")
                Zb = pool.tile([1, CHUNK * B], f32, tag="Zb")
                Sb = pool.tile([1, CHUNK * B], f32, tag="Sb")
                Ab = pool.tile([1, CHUNK * B], f32, tag="Ab")
                nc.sync.dma_start(Cb[:], Cd[r0:r0 + CHUNK, :])
                nc.sync.dma_start(Zb[:], Zd[r0:r0 + CHUNK, :])
                nc.sync.dma_start(Sb[:], Sd[r0:r0 + CHUNK, :])
                nc.sync.dma_start(Ab[:], Ad[r0:r0 + CHUNK, :])
                lstart = 0
                if ch == 0:
                    # step 0: p = softmax(start row 0) = spp row0 / kappa
                    nc.vector.tensor_scalar_mul(ph[:], Sb[0:1, 0:B],
                                                1.0 / KAPPA)
                    nc.vector.scalar_tensor_tensor(
                        jk[:], Ab[0:1, 0:B], 1.0, ph[:],
                        op0=OP.mult, op1=OP.mult, accum_out=dv[0:1, 0:1])
                    nc.vector.reciprocal(rr[:], dv[0:1, 0:1])
                    lstart = 1
                for l in range(lstart, CHUNK):
                    i = r0 + l
                    o = l * B
                    nc.vector.scalar_tensor_tensor(
                        jk[:], Zb[0:1, o:o + B], rr[0:1, 0:1], ph[:],
                        op0=OP.mult, op1=OP.mult, accum_out=mm[:])
                    nc.vector.scalar_tensor_tensor(
                        cq[:], Cb[0:1, o:o + B], rr[0:1, 0:1], ph[:],
                        op0=OP.mult, op1=OP.mult)
                    nc.vector.scalar_tensor_tensor(
                        ph[:], Sb[0:1, o:o + B], mm[0:1, 0:1], cq[:],
                        op0=OP.mult, op1=OP.add)
                    nc.vector.scalar_tensor_tensor(
                        jk[:], Ab[0:1, o:o + B], 1.0, ph[:],
                        op0=OP.mult, op1=OP.mult, accum_out=dv[0:1, i:i + 1])
                    nc.vector.reciprocal(rr[:], dv[0:1, i:i + 1])

            # ---- final: total = sum_j (T+1-j) ln d_j + ln(c_T . p / d_last)
            cT = cpool.tile([1, B], f32, tag="cT")
            nc.sync.dma_start(cT[:], Cd[T:T + 1, :])
            Fv = cpool.tile([1, 1], f32, tag="Fv")
            nc.vector.scalar_tensor_tensor(
                jk[:], cT[:], rr[0:1, 0:1], ph[:],
                op0=OP.mult, op1=OP.mult, accum_out=Fv[:])
            ld = cpool.tile([1, T], f32, tag="ld")
            nc.scalar.activation(ld[:], dv[:], AF.Ln)
            lF = cpool.tile([1, 1], f32, tag="lF")
            nc.scalar.activation(lF[:], Fv[:], AF.Ln)
            wi = cpool.tile([1, T], mybir.dt.int32, tag="wi")
            nc.gpsimd.iota(wi[:], pattern=[[-1, T]], base=T + 1,
                           channel_multiplier=0)
            wf = cpool.tile([1, T], f32, tag="wf")
            nc.vector.tensor_copy(wf[:], wi[:])
            wd = cpool.tile([1, T], f32, tag="wd")
            nc.vector.tensor_tensor(wd[:], ld[:], wf[:], op=OP.mult)
            S1 = cpool.tile([1, 1], f32, tag="S1")
            nc.vector.tensor_reduce(S1[:], wd[:], axis=AX.X, op=OP.add)
            tot = cpool.tile([1, 1], f32, tag="tot")
            nc.vector.tensor_tensor(tot[:], S1[:], lF[:], op=OP.add)
            nc.sync.dma_start(out[:], tot[:])
    nc.compile()
    return nc


def _make_runner():
    """Build the program once and wrap it in a persistent jitted callable so
    warm calls skip XLA re-trace/re-lowering (run_bass_kernel_spmd rebuilds
    its jit on every call, which costs >100 ms under axon)."""
    import jax
    from concourse import bass2jax as b2j
    from concourse import mybir

    nc = _build_program()
    b2j.install_neuronx_cc_hook()
    partition_name = (nc.partition_id_tensor.name
                      if nc.partition_id_tensor else None)
    in_names, out_names, out_avals, zero_outs = [], [], [], []
    for alloc in nc.m.functions[0].allocations:
        if not isinstance(alloc, mybir.MemoryLocationSet):
            continue
        name = alloc.memorylocations[0].name
        if alloc.kind == "ExternalInput":
            if name != partition_name:
                in_names.append(name)
        elif alloc.kind == "ExternalOutput":
            out_names.append(name)
            shape = tuple(alloc.tensor_shape)
            dtype = mybir.dt.np(alloc.dtype)
            out_avals.append(jax.core.ShapedArray(shape, dtype))
            zero_outs.append(np.zeros(shape, dtype))
    n_params = len(in_names)
    in_names_all = list(in_names) + out_names + (
        [partition_name] if partition_name else [])
    donate = tuple(range(n_params, n_params + len(out_avals)))

    def _body(*args):
        operands = list(args)
        if partition_name is not None:
            operands.append(b2j.partition_id_tensor())
        return tuple(b2j._bass_exec_p.bind(
            *operands, out_avals=tuple(out_avals),
            in_names=tuple(in_names_all), out_names=tuple(out_names),
            lowering_input_output_aliases=(), sim_require_finite=True,
            sim_require_nnan=True, nc=nc))

    jitted = jax.jit(_body, donate_argnums=donate, keep_unused=True)

    def run(in_map):
        args = [np.asarray(in_map[n]) for n in in_names]
        zeros = [np.zeros(z.shape, z.dtype) for z in zero_outs]
        outs = jitted(*args, *zeros)
        return {name: np.asarray(outs[i]) for i, name in enumerate(out_names)}

    return run


_cast8 = None


def kernel(s_i, W_action, W_stop, W_start, actions):
    global _runner, _cast8
    if _runner is None:
        _runner = _make_runner()
    if _cast8 is None:
        import jax
        hostcpu = jax.devices("cpu")[0]
        _cast8 = jax.jit(lambda x: x.astype(FP8), device=hostcpu)
    s = np.asarray(s_i, np.float32)
    Wcat = np.concatenate([np.asarray(W_action, np.float32),
                           np.asarray(W_stop, np.float32),
                           np.asarray(W_start, np.float32)], axis=1)
    s8 = np.asarray(_cast8(s))
    W8 = np.ascontiguousarray(Wcat * 64.0).astype(FP8)
    acts = np.asarray(actions).astype(np.int64).reshape(-1)
    OHm = np.zeros((T, A), BF16)
    OHm[np.arange(T), acts] = 1
    res = _runner({"s8": s8, "W8": W8, "OH": OHm})
    return np.float32(res["out"].reshape(-1)[0])
